# revision 33
# baseline (speedup 1.0000x reference)
"""Block-causal attention (B=2, S=2048, D=1024, H=16, HD=64, BLOCK=16) on 8 TRN2 cores.

Sharding: core c -> batch c//4, head-group c%4 (4 heads). Each core computes the
full attention for its 4 heads plus a partial out-projection y^T (1024, 2048);
the host sums the 4 partials per batch (row-parallel unshard) and transposes.

v2 layout (vs baseline): emission is pipelined so ACT (the exp bottleneck)
starts as early as possible — Q/K proj pair 0 -> V proj -> Q/K proj pair 1,
then attention per head-pair with the two heads interleaved so their K=64
score matmuls land on disjoint PE row-strips (tile_position (0,0)/(64,0))
and run concurrently; the rank-8 diag-mask matmuls get a second mu/mv copy
at partitions 64:72 for the same reason. RMS-norm ln/exp runs batched over
[4, 2048] (q+k rows of a pair) instead of 2x [2, 2048]. Q-side rrms
broadcast uses gpsimd partition_broadcast from SBUF (no DRAM roundtrip).
"""

import numpy as np
import ml_dtypes

import concourse.bass as bass
import concourse.tile as tile
from concourse import bacc
from concourse import mybir
from concourse.bass_utils import run_bass_kernel_spmd

BF16 = ml_dtypes.bfloat16
F32 = mybir.dt.float32
BF = mybir.dt.bfloat16

B, S, D, H, HD = 2, 2048, 1024, 16, 64
HLOC = 4          # heads per core
NCORES = 8
EPS = 1e-6
SCALE = HD ** -0.5
MASK_C = 8192.0   # masked-pair score offset; exp underflows to 0.0
NST = 4           # 512-wide seq tiles
NKT = 16          # 128-wide key tiles
NDK = 8           # 128-wide model-dim tiles


def _declare_io(nc):
    def din(name, shape, d=BF):
        return nc.dram_tensor(name, shape, d, kind="ExternalInput").ap()

    io = dict(
        xt_d=din("xt", [D, S]),
        wq_d=din("wq", [128, NDK * 256]),
        wk_d=din("wk", [128, NDK * 256]),
        wv_d=din("wv", [128, NDK * 256]),
        wo_d=din("wo", [128, 2 * D]),
        csq_d=din("csq", [128, S]),
        snq_d=din("snq", [128, S]),
        csk_d=din("csk", [128, S]),
        snk_d=din("snk", [128, S]),
        mu_d=din("mu", [8, 128]),
        mv_d=din("mv", [8, 128]),
        ones8_d=din("ones8", [128, 8]),
        b0_d=din("b0", [128, 1], F32),
        yt_d=nc.dram_tensor(
            "yt", [32, 128, 512], BF, kind="ExternalOutput"
        ).ap(),
    )
    return io


def _emit(tc, io, u=""):
    """Emit the per-core program. Pure SPMD: identical on all 8 cores."""
    from contextlib import ExitStack

    nc = tc.nc
    A = mybir.ActivationFunctionType
    xt_d = io["xt_d"]
    wq_d = io["wq_d"]
    wk_d = io["wk_d"]
    wv_d = io["wv_d"]
    wo_d = io["wo_d"]
    cs_d = {0: io["csq_d"], 1: io["csk_d"]}
    sn_d = {0: io["snq_d"], 1: io["snk_d"]}
    mu_d = io["mu_d"]
    mv_d = io["mv_d"]
    ones8_d = io["ones8_d"]
    b0_d = io["b0_d"]
    yt_d = io["yt_d"]

    ctx = ExitStack()
    proj_ctx = ExitStack()
    with ctx:
        consts = ctx.enter_context(tc.tile_pool(name="consts" + u, bufs=1))
        persist = ctx.enter_context(tc.tile_pool(name="persist" + u, bufs=1))
        dscratch = ctx.enter_context(
            tc.tile_pool(name="dscratch" + u, bufs=1, space="DRAM")
        )
        # score-psum pool opens first so its 4 banks are reserved from t=0:
        # attention scores (and the exp stream behind them) start on data
        # deps alone, while pair-1 projection still owns the other 4 banks.
        spp = ctx.enter_context(tc.tile_pool(name="spp" + u, bufs=1, space="PSUM"))
        # attention SBUF pools open before xtp/work2 so their addresses don't
        # overlap proj scratch (stack allocator) — pt tiles must be
        # allocatable while pair-1 projection still holds xt/work2.
        attnw = ctx.enter_context(tc.tile_pool(name="attnw" + u, bufs=1))
        ptp = ctx.enter_context(tc.tile_pool(name="ptp" + u, bufs=4))
        ystp = ctx.enter_context(tc.tile_pool(name="ystp" + u, bufs=3))
        vctx = ExitStack()
        xtp = vctx.enter_context(tc.tile_pool(name="xtp" + u, bufs=1))
        work2 = proj_ctx.enter_context(tc.tile_pool(name="work2" + u, bufs=2))
        sqp = proj_ctx.enter_context(tc.tile_pool(name="sqp" + u, bufs=3))
        pp = proj_ctx.enter_context(tc.tile_pool(name="pp" + u, bufs=2, space="PSUM"))
        msp = proj_ctx.enter_context(tc.tile_pool(name="msp" + u, bufs=1, space="PSUM"))

        # ---- input loads ----
        xt_sb = xtp.tile([128, NDK, S], BF)
        for kt in range(NDK):
            eng = nc.sync if kt % 2 == 0 else nc.scalar
            eng.dma_start(
                out=xt_sb[:, kt, :], in_=xt_d[128 * kt : 128 * (kt + 1), :]
            )
        wq_sb = consts.tile([128, NDK, 256], BF)
        wk_sb = consts.tile([128, NDK, 256], BF)
        wv_sb = consts.tile([128, NDK, 256], BF)
        wo_sb = consts.tile([128, 2, D], BF)
        nc.gpsimd.dma_start(out=wq_sb, in_=wq_d.rearrange("p (t m) -> p t m", t=NDK))
        nc.gpsimd.dma_start(out=wk_sb, in_=wk_d.rearrange("p (t m) -> p t m", t=NDK))
        cs_sb, sn_sb = {}, {}
        for qk in (0, 1):
            cs_sb[qk] = consts.tile([128, S], BF, name=f"cs{qk}")
            sn_sb[qk] = consts.tile([128, S], BF, name=f"sn{qk}")
            nc.gpsimd.dma_start(out=cs_sb[qk], in_=cs_d[qk])
            nc.gpsimd.dma_start(out=sn_sb[qk], in_=sn_d[qk])
        nc.gpsimd.dma_start(out=wv_sb, in_=wv_d.rearrange("p (t m) -> p t m", t=NDK))
        nc.gpsimd.dma_start(out=wo_sb, in_=wo_d.rearrange("p (t m) -> p t m", t=2))
        # dual-partition copies of the rank-8 mask factors so the two heads'
        # diag-mask matmuls use disjoint PE row strips (rows 0:8 / 64:72)
        msk_u = consts.tile([128, 128], BF)
        msk_v = consts.tile([128, 128], BF)
        nc.sync.dma_start(out=msk_u[0:8], in_=mu_d)
        nc.sync.dma_start(out=msk_u[64:72], in_=mu_d)
        nc.sync.dma_start(out=msk_v[0:8], in_=mv_d)
        nc.sync.dma_start(out=msk_v[64:72], in_=mv_d)
        ones8_sb = consts.tile([128, 8], BF)
        nc.sync.dma_start(out=ones8_sb, in_=ones8_d)
        b0_sb = consts.tile([128, 1], F32)
        nc.sync.dma_start(out=b0_sb, in_=b0_d)
        eps_sb = consts.tile([128, 1], F32)
        nc.vector.memset(eps_sb, EPS)

        # ---- persistent activations ----
        qT = persist.tile([128, 2, S], BF)
        kT = persist.tile([128, 2, S], BF)
        vv = persist.tile([128, NKT, HLOC, HD + 1], BF)   # [V | ones]
        at = persist.tile([128, 2, S], BF)      # normalized attn^T
        ln8 = persist.tile([98, NST, 512], BF)
        rr8 = persist.tile([98, NST, 512], BF)
        rkb = persist.tile([128, 64], BF)
        rkz = persist.tile([128, 4, 16], F32)
        rr_dram = dscratch.tile([8, 16, 128], BF)

        nc.vector.memset(vv[:, :, :, HD : HD + 1], 1.0)

        wsb = {0: wq_sb, 1: wk_sb}

        def rope_side(qk, raw_t, dest_write, engs=(None, None)):
            """rotate-half via 2 partition-permuted DMAs + cos/sin muls.
            dest_write(tsum_ap) consumes the rotated+summed result."""
            engs = engs if engs[0] is not None else (nc.sync, nc.scalar)
            rot = work2.tile([128, S], BF, tag=f"rot{qk}", name=f"rot{qk}", bufs=1)
            for bi, (lo, hi) in enumerate(((0, 32), (32, 64), (64, 96), (96, 128))):
                src_lo = lo + 32 if (lo // 32) % 2 == 0 else lo - 32
                engs[bi % 2].dma_start(out=rot[lo:hi], in_=raw_t[src_lo : src_lo + 32])
            t1 = work2.tile([128, S], BF, tag="t1", bufs=1)
            t2 = work2.tile([128, S], BF, tag="t2", bufs=1)
            nc.vector.tensor_mul(t1, raw_t, cs_sb[qk])
            nc.vector.tensor_mul(t2, rot, sn_sb[qk])
            dest_write(t1, t2)

        def proj_side(qk, mt, ms_tiles, batched):
            """projection matmuls for one side (q or k) of pair mt; the ms
            sums go to rows 0:2 (split, per-side groups) or rows 0:2/2:4 of
            shared tiles (batched, one accum group across q+k)."""
            raw_t = work2.tile([128, S], BF, tag=f"raw{qk}", name=f"raw{qk}", bufs=1)
            for st in range(NST):
                ps = pp.tile([128, 512], F32, tag="pp")
                for kt in range(NDK):
                    nc.tensor.matmul(
                        ps,
                        lhsT=wsb[qk][:, kt, 128 * mt : 128 * (mt + 1)],
                        rhs=xt_sb[:, kt, 512 * st : 512 * (st + 1)],
                        start=(kt == 0),
                        stop=(kt == NDK - 1),
                    )
                sl = slice(512 * st, 512 * (st + 1))
                nc.vector.tensor_copy(raw_t[:, sl], ps)
                sq = sqp.tile([128, 512], BF, tag="sq")
                nc.vector.tensor_mul(sq, raw_t[:, sl], raw_t[:, sl])
                if batched:
                    lhs_ms = ones8_sb[:, 4 * qk : 4 * qk + 4]
                    st_fl, sp_fl = (qk == 0), (qk == 1)
                else:
                    lhs_ms = ones8_sb[:, 6 * qk : 6 * qk + 2]
                    st_fl, sp_fl = True, True
                nc.tensor.matmul(
                    ms_tiles[st // 2][0 : 4 if batched else 2, st % 2, :],
                    lhsT=lhs_ms,
                    rhs=sq,
                    start=st_fl,
                    stop=sp_fl,
                )
            return raw_t

        def rms_ln(ms_tiles, ln_row, nrow):
            """ln+exp over nrow rows of the ms tiles -> rr8[ln_row:+nrow]."""
            for h in range(2):
                nc.scalar.activation(
                    ln8[ln_row : ln_row + nrow, 2 * h : 2 * h + 2],
                    ms_tiles[h][0:nrow],
                    A.Ln,
                    bias=eps_sb[0:nrow],
                    scale=1.0 / HD,
                )
            nc.scalar.activation(
                rr8[ln_row : ln_row + nrow], ln8[ln_row : ln_row + nrow], A.Exp,
                scale=-0.5,
            )

        def q_chain(mt, ln_row, t1, t2):
            """rrms broadcast across each head's 64 partitions + qT write."""
            qe = nc.scalar if mt == 0 else nc.sync
            tsum = work2.tile([128, S], BF, tag="tsum", bufs=1)
            nc.vector.tensor_add(tsum, t1, t2)
            qe.dma_start(
                out=rr_dram[2 * mt : 2 * mt + 2].rearrange("r a b -> r (a b)"),
                in_=rr8[ln_row : ln_row + 2].rearrange("p a b -> p (a b)"),
            )
            rrb = work2.tile([128, NST, 512], BF, tag="rrb", bufs=1)
            for j in (0, 1):
                qe.dma_start(
                    out=rrb[64 * j : 64 * (j + 1)],
                    in_=rr_dram[2 * mt + j : 2 * mt + j + 1]
                    .rearrange("r a b -> r (a b)")
                    .rearrange("r (a b) -> r a b", a=NST)
                    .partition_broadcast(64),
                )
            for st in range(NST):
                sl = slice(512 * st, 512 * (st + 1))
                nc.vector.tensor_mul(qT[:, mt, sl], tsum[:, sl], rrb[:, st, :])

        def k_chain(mt, ln_row, t1, t2):
            """kT write + per-key-partition exp scales (via DRAM transpose)."""
            nc.vector.tensor_add(kT[:, mt, :], t1, t2)
            nc.sync.dma_start(
                out=rr_dram[4 + 2 * mt : 6 + 2 * mt].rearrange("r a b -> r (a b)"),
                in_=rr8[ln_row : ln_row + 2].rearrange("p a b -> p (a b)"),
            )
            nc.sync.dma_start_transpose(
                rkb[:, 32 * mt : 32 * (mt + 1)],
                rr_dram[4 + 2 * mt : 6 + 2 * mt].rearrange("r a b -> (r a) b"),
            )
            nc.vector.tensor_scalar_mul(
                rkz[:, 2 * mt : 2 * mt + 2, :].rearrange("p h i -> p (h i)"),
                rkb[:, 32 * mt : 32 * (mt + 1)],
                SCALE,
            )

        def proj_pair(mt):
            """pair 0: per-side split rms (early chains). pair 1: q+k share
            ms tiles, one batched 4-row ln/exp (less mid-stream ACT work)."""
            if mt == 0:
                for qk in (0, 1):
                    ms_tiles = [
                        msp.tile([4, 2, 512], F32, tag="ms", name=f"ms{qk}{h}")
                        for h in range(2)
                    ]
                    raw_t = proj_side(qk, 0, ms_tiles, batched=False)
                    rms_ln(ms_tiles, 32 * qk, 2)
                    if qk == 0:
                        rope_side(0, raw_t, lambda t1, t2: q_chain(0, 0, t1, t2))
                    else:
                        rope_side(1, raw_t, lambda t1, t2: k_chain(0, 32, t1, t2))
            else:
                ms_tiles = [
                    msp.tile([4, 2, 512], F32, tag="ms", name=f"msb{h}")
                    for h in range(2)
                ]
                raw_q = proj_side(0, 1, ms_tiles, batched=True)
                raw_k = proj_side(1, 1, ms_tiles, batched=True)
                rms_ln(ms_tiles, 64, 4)
                engs = (nc.sync, nc.sync)
                rope_side(0, raw_q, lambda t1, t2: q_chain(1, 64, t1, t2), engs)
                rope_side(1, raw_k, lambda t1, t2: k_chain(1, 66, t1, t2), engs)

        def v_proj(st_range):
            for st in st_range:
                ps = pp.tile([128, 512], F32, tag="pp", name="vps")
                for kt in range(NDK):
                    nc.tensor.matmul(
                        ps[:, 0:256],
                        lhsT=xt_sb[:, kt, 128 * st : 128 * (st + 1)],
                        rhs=wv_sb[:, kt, :],
                        start=(kt == 0),
                        stop=(kt == NDK - 1),
                    )
                nc.vector.tensor_copy(
                    vv[:, st, :, 0:HD],
                    ps[:, 0:256].rearrange("p (h d) -> p h d", h=HLOC),
                )

        proj_pair(0)
        v_proj(range(NKT))
        proj_pair(1)
        proj_ctx.close()
        vctx.close()
        avp = ctx.enter_context(tc.tile_pool(name="avp" + u, bufs=1, space="PSUM"))

        # ---- attention, qh-major: both head-pairs' qh0, then the jh=0
        # out-projection (overlapping qh1's attention), then qh1, jh=1 ----
        def attn_qh(mt, qh):
            glo = 1024 * qh
            kmax = 8 * (qh + 1)
            av = {hh: avp.tile([65, 2, 512], F32, tag=f"av{hh}", name=f"av{hh}") for hh in (0, 1)}
            for i in range(kmax):
                q0 = 128 * i
                lo_g = max(glo, q0)
                has_diag = glo <= q0 < glo + 1024
                pt = {}
                sp = {}
                for hh in (0, 1):
                    po = 64 * hh
                    sp[hh] = spp.tile([128, 1024], F32, tag=f"sp{hh}", name=f"sp{hh}")
                    for jj in range(2):
                        j = 2 * qh + jj
                        lo = max(512 * j, q0)
                        hi = 512 * (j + 1)
                        if lo >= hi:
                            continue
                        diag_bank = has_diag and (q0 - glo) // 512 == jj
                        nc.tensor.matmul(
                            sp[hh][:, lo - glo : hi - glo],
                            lhsT=kT[po : po + 64, mt, 128 * i : 128 * (i + 1)],
                            rhs=qT[po : po + 64, mt, lo:hi],
                            start=True,
                            stop=not diag_bank,
                        )
                        if diag_bank:
                            nc.tensor.matmul(
                                sp[hh][:, q0 - glo : q0 - glo + 128],
                                lhsT=msk_u[po : po + 8, :],
                                rhs=msk_v[po : po + 8, :],
                                start=False,
                                stop=True,
                            )
                for hh in (0, 1):
                    h = 2 * mt + hh
                    pt[hh] = ptp.tile([128, 1024], BF, tag=f"pt{hh}", name=f"pt{hh}", bufs=9)
                    nc.scalar.activation(
                        pt[hh][:, lo_g - glo : 1024],
                        sp[hh][:, lo_g - glo : 1024],
                        A.Exp,
                        bias=b0_sb,
                        scale=rkz[:, h, i : i + 1],
                    )
                for hh in (0, 1):
                    h = 2 * mt + hh
                    for jj in range(2):
                        j = 2 * qh + jj
                        jlo = max(512 * j, q0)
                        jhi = 512 * (j + 1)
                        if jlo >= jhi:
                            continue
                        nc.tensor.matmul(
                            av[hh][:, jj, jlo - 512 * j : 512],
                            lhsT=vv[:, i, h, :],
                            rhs=pt[hh][:, jlo - glo : jhi - glo],
                            start=(i == 0),
                            stop=(i == min(kmax, 4 * j + 4) - 1),
                        )
            for hh in (0, 1):
                po = 64 * hh
                rden = attnw.tile([1, 2, 512], F32, tag=f"rden{hh}", name=f"rden{hh}")
                nc.vector.reciprocal(rden, av[hh][64:65])
                rdb = attnw.tile([64, 2, 512], F32, tag=f"rdb{hh}")
                nc.gpsimd.partition_broadcast(rdb, rden, channels=64)
                for jj in range(2):
                    nc.vector.tensor_mul(
                        at[
                            po : po + 64,
                            mt,
                            glo + 512 * jj : glo + 512 * (jj + 1),
                        ],
                        av[hh][0:64, jj, :],
                        rdb[:, jj, :],
                    )

        def p4_jpair(jh):
            """out-projection for query blocks j = 2jh, 2jh+1."""
            for m in range(8):
                ps = spp.tile([128, 1024], F32, tag=f"sp{m % 2}", name="ps4")
                for kt in range(2):
                    for jj in range(2):
                        j = 2 * jh + jj
                        nc.tensor.matmul(
                            ps[:, 512 * jj : 512 * (jj + 1)],
                            lhsT=wo_sb[:, kt, 128 * m : 128 * (m + 1)],
                            rhs=at[:, kt, 512 * j : 512 * (j + 1)],
                            start=(kt == 0),
                            stop=(kt == 1),
                        )
                yst = ystp.tile([128, 2, 512], BF, tag="yst")
                pv = ps.rearrange("p (a b) -> p a b", a=2)
                nc.vector.tensor_copy(yst[:, 0], pv[:, 0])
                nc.scalar.copy(yst[:, 1], pv[:, 1])
                eng = (nc.sync, nc.scalar)[m % 2]
                eng.dma_start(
                    out=yt_d[4 * m + 2 * jh : 4 * m + 2 * jh + 2].rearrange(
                        "a p b -> p a b"
                    ),
                    in_=yst,
                )

        # attention outranks the V projection and pair-1 proj on the PE so
        # the ACT exp stream (the kernel bottleneck) starts as early as
        # possible; V fills PE gaps and av matmuls catch up later.
        with tc.high_priority(offset=520):
            attn_qh(0, 0)
            attn_qh(0, 1)
            attn_qh(1, 0)
            attn_qh(1, 1)
            p4_jpair(0)
            p4_jpair(1)

class _pin_act_table:
    """Force every activation we use (Exp, Ln, Copy) onto the one table set
    containing them all, so the program does a single ACT_TABLE_LOAD."""

    def __init__(self, arch):
        from concourse.hw_specs import get_activation_tables

        self.tabs = get_activation_tables(arch)

    def __enter__(self):
        self.saved = {nm: set(s) for nm, s in self.tabs.items()}
        for nm, s in self.tabs.items():
            if nm != "natural_log_exp_and_others":
                s.clear()

    def __exit__(self, *a):
        for nm, s in self.tabs.items():
            s.clear()
            s.update(self.saved[nm])


def build_program(iters=1):
    nc = bacc.Bacc(
        "TRN2",
        target_bir_lowering=False,
        debug=False,
        enable_asserts=False,
        num_devices=NCORES,
    )
    with tile.TileContext(nc) as tc:
        io = _declare_io(nc)
        for it in range(iters):
            _emit(tc, io, u=f"_i{it}" if iters > 1 else "")
    with _pin_act_table(nc.m.arch):
        nc.compile()
    return nc


def make_core_inputs(x, qkv_w, out_w, qn_w, kn_w, rope_cos, rope_sin, attention_mask):
    """Host-side shard/layout prep. Returns list of 8 per-core input dicts."""
    x = np.asarray(x, np.float32)
    qkv_w = np.asarray(qkv_w, np.float32)
    out_w = np.asarray(out_w, np.float32)
    qn_w = np.asarray(qn_w, np.float32)
    kn_w = np.asarray(kn_w, np.float32)
    rope_cos = np.asarray(rope_cos, np.float32)
    rope_sin = np.asarray(rope_sin, np.float32)
    am = np.asarray(attention_mask)

    r = qkv_w.reshape(3, H, HD, D)
    csT = rope_cos.T.astype(np.float32)                # (64, S)
    snT = rope_sin.T.astype(np.float32)
    s2 = np.concatenate([-snT[0:32], snT[32:64]], axis=0)  # sign-folded sin
    perm = np.concatenate([np.arange(32, 64), np.arange(0, 32)])

    def fold(tab, w, permute):
        ww = w[perm] if permute else w
        t = tab * ww[:, None]
        return np.concatenate([t, t], axis=0).astype(BF16)  # (128, S)

    csq = fold(csT, qn_w, False)
    snq = fold(s2, qn_w, True)
    csk = fold(csT, kn_w, False)
    snk = fold(s2, kn_w, True)

    # rank-8 factorization of the (128,128) diagonal-block mask
    dis = ~(am[0:128, 0:128].T)                        # dis[k', q'] disallowed
    mu = np.zeros((8, 128), np.float32)
    mv = np.zeros((8, 128), np.float32)
    for t in range(8):
        mu[t] = np.arange(128) // 16 == t
        mv[t] = -MASK_C * dis[16 * t, :]
    # ms-matmul lhsT: cols 0:2 = q head indicators (rows 0:2 of ms),
    # cols 2:4 zero; cols 4:6 zero, cols 6:8 = k head indicators (rows 2:4)
    ones8 = np.zeros((128, 8), np.float32)
    ones8[0:64, 0] = 1.0
    ones8[64:128, 1] = 1.0
    ones8[0:64, 6] = 1.0
    ones8[64:128, 7] = 1.0
    b0 = float(HD * SCALE * max(1e-30, np.abs(qn_w).max() * np.abs(kn_w).max()))
    b0_t = np.full((128, 1), -b0, np.float32)

    shared = dict(
        csq=csq,
        snq=snq,
        csk=csk,
        snk=snk,
        mu=mu.astype(BF16),
        mv=mv.astype(BF16),
        ones8=ones8.astype(BF16),
        b0=b0_t,
    )
    in_maps = []
    for c in range(NCORES):
        b, g = divmod(c, 4)
        hs = slice(HLOC * g, HLOC * (g + 1))
        m = dict(shared)
        m["xt"] = np.ascontiguousarray(x[b].T).astype(BF16)

        def _wlayout(w):
            mm = w.shape[1]
            return np.ascontiguousarray(
                w.reshape(-1, 128, mm).transpose(1, 0, 2).reshape(128, -1)
            ).astype(BF16)

        m["wq"] = _wlayout(r[0, hs].transpose(2, 0, 1).reshape(D, 256))
        m["wk"] = _wlayout(r[1, hs].transpose(2, 0, 1).reshape(D, 256))
        m["wv"] = _wlayout(r[2, hs].transpose(2, 0, 1).reshape(D, 256))
        m["wo"] = _wlayout(
            np.ascontiguousarray(out_w[:, 256 * g : 256 * (g + 1)].T)
        )
        in_maps.append(m)
    return in_maps


_PROGRAM = []


def get_program():
    if not _PROGRAM:
        _PROGRAM.append(build_program())
    return _PROGRAM[0]


def unshard(results):
    """results: list of 8 dicts with 'yt' (32, 128, 512) fp32 partials."""
    ys = []
    for b in range(B):
        acc = np.zeros((32, 128, 512), np.float64)
        for g in range(4):
            acc += np.asarray(results[4 * b + g]["yt"], np.float32)
        yt = acc.reshape(8, 4, 128, 512).transpose(0, 2, 1, 3).reshape(D, S)
        ys.append(yt.T.astype(np.float32))
    return np.stack(ys)


def kernel(**inputs):
    in_maps = make_core_inputs(**inputs)
    nc = get_program()
    res = run_bass_kernel_spmd(nc, in_maps, core_ids=list(range(NCORES)))
    return unshard(res.results)
